# revision 1
# baseline (speedup 1.0000x reference)
"""GroupedQueryAttention forward on 8 Trainium2 NeuronCores (Bass/Tile).

Sharding (per spec hint): data-parallel over batch (B=2) x tensor-parallel
over KV-head groups (4 groups of 2 KV heads + their 8 query heads each).
Core c -> (batch b = c // 4, group g = c % 4).

Each core computes, for its batch element and its 8 query heads:
  qT/kT projections in transposed layout (lhsT = W, rhs = xT), V natural via
  on-chip PE transpose of vT; causal softmax without max-subtraction (scores
  are ~N(0,1) after the 1/sqrt(hd) scale, exp cannot overflow); the softmax
  denominator is produced by the same matmul as attn@V via a ones-column
  appended to V; normalization is folded into the o_proj stationary tiles.
  o_proj is row-parallel: each core emits a full [N, D] fp32 partial, and the
  host sums the 4 partials per batch element (the "all-reduce" of the o_proj).

All device compute is bf16 with fp32 PSUM accumulation. The host pre-casts
and pre-transposes x (xT) and pre-slices/reorders the weight shards so the
device performs no layout work on the inputs.
"""

import numpy as np

import concourse.bass as bass  # noqa: F401  (import keeps engine registry warm)
import concourse.mybir as mybir
import concourse.tile as tile
from concourse import bacc, bass_utils

# Problem shape (hardcoded per contract).
B, N, D = 2, 2048, 2048
NUM_HEADS = 32
NUM_KV_HEADS = 8
HD = 64                      # head dim
G = NUM_HEADS // NUM_KV_HEADS  # 4 query heads per kv head
N_CORES = 8
LQ = 8                       # local query heads per core (2 kv heads * G)
NT = D // 128                # 16 contraction tiles
NCHUNK = 4                   # token chunks of 512
CH = 512

_CACHE = {}


def _build():
    nc = bacc.Bacc("TRN2", target_bir_lowering=False, debug=False,
                   num_devices=N_CORES)
    f32, bf16 = mybir.dt.float32, mybir.dt.bfloat16

    xT = nc.dram_tensor("xT", [D, N], bf16, kind="ExternalInput")
    wq = nc.dram_tensor("wq", [D, 512], bf16, kind="ExternalInput")
    wk = nc.dram_tensor("wk", [D, 128], bf16, kind="ExternalInput")
    wv = nc.dram_tensor("wv", [D, 128], bf16, kind="ExternalInput")
    wo = nc.dram_tensor("wo", [512, D], bf16, kind="ExternalInput")
    msk = nc.dram_tensor("msk", [128, 4 * CH], bf16, kind="ExternalInput")
    iden = nc.dram_tensor("iden", [128, 128], bf16, kind="ExternalInput")
    sel = nc.dram_tensor("sel", [8, 4 * 128], f32, kind="ExternalInput")
    part = nc.dram_tensor("part", [N, D], f32, kind="ExternalOutput")

    with tile.TileContext(nc) as tc:
        with (
            tc.tile_pool(name="const", bufs=1) as cpool,
            tc.tile_pool(name="proj", bufs=1) as ppool,
            tc.tile_pool(name="work", bufs=4) as wpool,
            tc.tile_pool(name="att", bufs=1) as apool,
            tc.tile_pool(name="stage", bufs=3) as spool,
            tc.tile_pool(name="ps_s", bufs=2, space="PSUM") as ps_s,
            tc.tile_pool(name="ps_av", bufs=4, space="PSUM") as ps_av,
            tc.tile_pool(name="ps_m", bufs=1, space="PSUM") as ps_m,
        ):
            # ---- load constants / inputs to SBUF -------------------------
            xt = cpool.tile([128, NT * N], bf16, tag="xt")
            nc.sync.dma_start(
                xt[:].rearrange("p (t n) -> p t n", t=NT),
                xT.ap().rearrange("(t p) n -> p t n", p=128))
            wq_t = cpool.tile([128, NT * 512], bf16, tag="wq")
            nc.sync.dma_start(
                wq_t[:].rearrange("p (t o) -> p t o", t=NT),
                wq.ap().rearrange("(t p) o -> p t o", p=128))
            wk_t = cpool.tile([128, NT * 128], bf16, tag="wk")
            nc.sync.dma_start(
                wk_t[:].rearrange("p (t o) -> p t o", t=NT),
                wk.ap().rearrange("(t p) o -> p t o", p=128))
            wv_t = cpool.tile([128, NT * 128], bf16, tag="wv")
            nc.sync.dma_start(
                wv_t[:].rearrange("p (t o) -> p t o", t=NT),
                wv.ap().rearrange("(t p) o -> p t o", p=128))
            wo_t = cpool.tile([128, 4 * D], bf16, tag="wo")
            nc.sync.dma_start(
                wo_t[:].rearrange("p (t o) -> p t o", t=4),
                wo.ap().rearrange("(t p) o -> p t o", p=128))
            msk_t = cpool.tile([128, 4 * CH], bf16, tag="msk")
            nc.sync.dma_start(msk_t[:], msk.ap()[:])
            id_t = cpool.tile([128, 128], bf16, tag="iden")
            nc.sync.dma_start(id_t[:], iden.ap()[:])
            ones64 = cpool.tile([1, 64], f32, tag="ones64")
            nc.vector.memset(ones64[:], 1.0)
            sel_t = cpool.tile([8, 4 * 128], f32, tag="sel")
            nc.sync.dma_start(sel_t[:], sel.ap()[:])

            # ---- projections --------------------------------------------
            # kT2 [128 (2 kv heads x 64), N]
            kt2 = ppool.tile([128, N], bf16, tag="kt2")
            for j in range(N // CH):
                ps = ps_m.tile([128, CH], f32, tag="misc")
                for t in range(NT):
                    nc.tensor.matmul(
                        ps[:], wk_t[:, t * 128:(t + 1) * 128],
                        xt[:, t * N + j * CH: t * N + (j + 1) * CH],
                        start=(t == 0), stop=(t == NT - 1))
                nc.scalar.activation(kt2[:, j * CH:(j + 1) * CH], ps[:],
                                     mybir.ActivationFunctionType.Copy)
            # vT [128, N] then transpose to V3 [128, 16*130] (V + ones col)
            v3 = apool.tile([128, 16 * 130], bf16, tag="v3")
            nc.vector.memset(v3[:], 1.0)
            for j in range(N // CH):
                ps = ps_m.tile([128, CH], f32, tag="misc")
                for t in range(NT):
                    nc.tensor.matmul(
                        ps[:], wv_t[:, t * 128:(t + 1) * 128],
                        xt[:, t * N + j * CH: t * N + (j + 1) * CH],
                        start=(t == 0), stop=(t == NT - 1))
                vt_s = spool.tile([128, CH], bf16, tag="vt")
                nc.scalar.activation(vt_s[:], ps[:],
                                     mybir.ActivationFunctionType.Copy)
                for s in range(4):       # 4 m-tiles of 128 in this chunk
                    mt = 4 * j + s
                    pst = ps_m.tile([128, 128], bf16, tag="tr")
                    nc.tensor.transpose(pst[:], vt_s[:, s * 128:(s + 1) * 128],
                                        id_t[:])
                    nc.vector.tensor_copy(v3[:, mt * 130: mt * 130 + 64],
                                          pst[:, 0:64])
                    nc.vector.tensor_copy(v3[:, mt * 130 + 65: mt * 130 + 129],
                                          pst[:, 64:128])
            # qT2 chunks a=0..3: [128 (head a | head a+4), N]
            qt2 = []
            for a in range(4):
                qa = ppool.tile([128, N], bf16, tag=f"qt2_{a}")
                for j in range(N // CH):
                    ps = ps_m.tile([128, CH], f32, tag="misc")
                    for t in range(NT):
                        nc.tensor.matmul(
                            ps[:], wq_t[:, t * 512 + a * 128: t * 512 + (a + 1) * 128],
                            xt[:, t * N + j * CH: t * N + (j + 1) * CH],
                            start=(t == 0), stop=(t == NT - 1))
                    nc.scalar.activation(qa[:, j * CH:(j + 1) * CH], ps[:],
                                         mybir.ActivationFunctionType.Copy)
                qt2.append(qa)

            # ---- attention + o_proj per token chunk ---------------------
            for ci in range(NCHUNK):
                n0 = ci * CH
                mt_hi = 4 * ci + 4          # m-tiles 0..mt_hi-1
                aot = []                     # attn_outT tiles per pair
                sum8 = apool.tile([1, 8 * CH], f32, tag="sum8")
                for wave in range(2):
                    for a in (2 * wave, 2 * wave + 1):
                        pa0 = ps_av.tile([128, CH], f32, tag="av")
                        pa1 = ps_av.tile([128, CH], f32, tag="av")
                        for mt in range(mt_hi):
                            diag = mt - 4 * ci
                            ss0 = ps_s.tile([128, CH], f32, tag="s")
                            ss1 = ps_s.tile([128, CH], f32, tag="s")
                            nc.tensor.matmul(
                                ss0[:], kt2[0:64, mt * 128:(mt + 1) * 128],
                                qt2[a][0:64, n0:n0 + CH],
                                start=True, stop=True)
                            nc.tensor.matmul(
                                ss1[:], kt2[64:128, mt * 128:(mt + 1) * 128],
                                qt2[a][64:128, n0:n0 + CH],
                                start=True, stop=True)
                            pt0 = wpool.tile([128, CH], bf16, tag="pt")
                            pt1 = wpool.tile([128, CH], bf16, tag="pt")
                            nc.scalar.activation(
                                pt0[:], ss0[:],
                                mybir.ActivationFunctionType.Exp, scale=0.125)
                            nc.scalar.activation(
                                pt1[:], ss1[:],
                                mybir.ActivationFunctionType.Exp, scale=0.125)
                            if diag >= 0:
                                mslc = msk_t[:, diag * CH:(diag + 1) * CH]
                                nc.vector.tensor_mul(pt0[:], pt0[:], mslc)
                                nc.vector.tensor_mul(pt1[:], pt1[:], mslc)
                            nc.tensor.matmul(
                                pa0[0:65, :], v3[:, mt * 130: mt * 130 + 65],
                                pt0[:], start=(mt == 0), stop=(mt == mt_hi - 1))
                            nc.tensor.matmul(
                                pa1[0:65, :], v3[:, mt * 130 + 65: mt * 130 + 130],
                                pt1[:], start=(mt == 0), stop=(mt == mt_hi - 1))
                        ao = apool.tile([128, CH], bf16, tag=f"ao_{a}")
                        nc.vector.tensor_copy(ao[0:64, :], pa0[0:64, :])
                        nc.vector.tensor_copy(ao[64:128, :], pa1[0:64, :])
                        nc.vector.tensor_copy(sum8[0:1, a * CH:(a + 1) * CH], pa0[64:65, :])
                        nc.vector.tensor_copy(sum8[0:1, (a + 4) * CH:(a + 5) * CH], pa1[64:65, :])
                        aot.append(ao)
                aos = []
                for a in range(4):
                    rb = ps_m.tile([128, CH], f32, tag="misc")
                    nc.tensor.matmul(rb[0:64, :], ones64[0:1, :],
                                     sum8[0:1, a * CH:(a + 1) * CH],
                                     start=True, stop=True, tile_position=(0, 0))
                    nc.tensor.matmul(rb[64:128, :], ones64[0:1, :],
                                     sum8[0:1, (a + 4) * CH:(a + 5) * CH],
                                     start=True, stop=True, tile_position=(0, 64))
                    rbr = spool.tile([128, CH], f32, tag="rbr")
                    nc.vector.reciprocal(rbr[:], rb[:])
                    an = apool.tile([128, CH], bf16, tag=f"aos_{a}")
                    nc.vector.tensor_mul(an[:], aot[a][:], rbr[:])
                    aos.append(an)
                # o_proj: out[n, :] += sum_c attn_outT_s[c, n] * Wo[c, :]
                for nt in range(4):
                    for dc in range(4):
                        po = ps_m.tile([128, CH], f32, tag="misc")
                        for a in range(4):
                            nc.tensor.matmul(
                                po[:], aos[a][:, nt * 128:(nt + 1) * 128],
                                wo_t[:, a * D + dc * CH: a * D + (dc + 1) * CH],
                                start=(a == 0), stop=(a == 3))
                        st = spool.tile([128, CH], f32, tag="ost")
                        nc.vector.tensor_copy(st[:], po[:])
                        nc.sync.dma_start(
                            part.ap()[n0 + nt * 128: n0 + (nt + 1) * 128,
                                      dc * CH:(dc + 1) * CH],
                            st[:])
    nc.compile()
    return nc


def _prep_in_maps(x, Wq, Wk, Wv, Wo):
    import jax.numpy as jnp

    def to_bf16(a):
        return np.asarray(jnp.asarray(np.asarray(a), dtype=jnp.bfloat16))

    # causal mask tiles for diagonal offsets 0..3 (within a 512 chunk)
    msk = np.zeros((128, 4 * CH), np.float32)
    for k in range(4):
        i = np.arange(128)[:, None]
        j = np.arange(CH)[None, :]
        msk[:, k * CH:(k + 1) * CH] = (i + 128 * k <= j).astype(np.float32)
    iden = np.eye(128, dtype=np.float32)
    sel = np.zeros((8, 4 * 128), np.float32)
    for a in range(4):
        sel[a, a * 128: a * 128 + 64] = 1.0
        sel[a + 4, a * 128 + 64: (a + 1) * 128] = 1.0

    in_maps = []
    for c in range(N_CORES):
        b, g = c // 4, c % 4
        qh = [8 * g + a for a in range(8)]      # global q heads for this core
        # Wq columns reordered into pair chunks [head a | head a+4]
        wq_cols = []
        for a in range(4):
            wq_cols.append(np.arange(qh[a] * HD, (qh[a] + 1) * HD))
            wq_cols.append(np.arange(qh[a + 4] * HD, (qh[a + 4] + 1) * HD))
        wq_r = np.asarray(Wq)[:, np.concatenate(wq_cols)]
        wo_rows = wq_cols  # same ordering for Wo rows
        wo_r = np.asarray(Wo)[np.concatenate(wo_rows), :]
        wk_s = np.asarray(Wk)[:, 2 * g * HD: (2 * g + 2) * HD]
        wv_s = np.asarray(Wv)[:, 2 * g * HD: (2 * g + 2) * HD]
        in_maps.append({
            "xT": to_bf16(np.asarray(x)[b].T),
            "wq": to_bf16(wq_r),
            "wk": to_bf16(wk_s),
            "wv": to_bf16(wv_s),
            "wo": to_bf16(wo_r),
            "msk": to_bf16(msk),
            "iden": to_bf16(iden),
            "sel": sel,
        })
    return in_maps


def kernel(x, Wq, Wk, Wv, Wo, trace=False):
    if "nc" not in _CACHE:
        _CACHE["nc"] = _build()
    nc = _CACHE["nc"]
    in_maps = _prep_in_maps(x, Wq, Wk, Wv, Wo)
    res = bass_utils.run_bass_kernel_spmd(
        nc, in_maps, core_ids=list(range(N_CORES)), trace=trace)
    _CACHE["last_result"] = res
    out = np.zeros((B, N, D), np.float32)
    for c in range(N_CORES):
        out[c // 4] += res.results[c]["part"]
    return out



# revision 2
# speedup vs baseline: 1.5364x; 1.5364x over previous
"""GroupedQueryAttention forward on 8 Trainium2 NeuronCores (Bass/Tile).

Sharding (per spec hint): data-parallel over batch (B=2) x tensor-parallel
over KV-head groups (4 groups of 2 KV heads + their 8 query heads each).
Core c -> (batch b = c // 4, group g = c % 4).

Each core computes, for its batch element and its 8 query heads:
  qT/kT projections in transposed layout (lhsT = W, rhs = xT), V natural via
  on-chip PE transpose of vT; causal softmax without max-subtraction (scores
  are ~N(0,1) after the 1/sqrt(hd) scale, exp cannot overflow); the softmax
  denominator is produced by the same matmul as attn@V via a ones-column
  appended to V. o_proj is row-parallel: each core emits a full [N, D] fp32
  partial, and the host sums the 4 partials per batch element.

v2 structure (vs the v1 baseline):
  - per-chunk interleave: kv proj, then per 512-token chunk
    qproj(ci) -> attention(ci) -> [qproj(ci+1) overlap] -> o_proj(ci)
  - per (pair, mt): both kv-heads' score matmuls land in one 2-bank PSUM
    tile -> a single batched Exp; causal-diagonal tiles trim the dead
    query range out of scores/exp/attnV; mask multiply shrinks to the
    128x128 triangular block.
  - softmax denominators: reciprocal_approx_fast + GpSimd partition
    broadcast (replaces fp32 PE broadcast matmuls of v1).
All device compute is bf16 with fp32 PSUM accumulation.
"""

import numpy as np

import concourse.bass as bass  # noqa: F401  (import keeps engine registry warm)
import concourse.mybir as mybir
import concourse.tile as tile
from concourse import bacc, bass_utils

# Problem shape (hardcoded per contract).
B, N, D = 2, 2048, 2048
NUM_HEADS = 32
NUM_KV_HEADS = 8
HD = 64
G = NUM_HEADS // NUM_KV_HEADS
N_CORES = 8
NT = D // 128                # 16 contraction tiles
CH = 512
NCHUNK = N // CH             # 4

_CACHE = {}


def _build():
    nc = bacc.Bacc("TRN2", target_bir_lowering=False, debug=False,
                   num_devices=N_CORES)
    f32, bf16 = mybir.dt.float32, mybir.dt.bfloat16
    Copy = mybir.ActivationFunctionType.Copy
    Exp = mybir.ActivationFunctionType.Exp

    xT = nc.dram_tensor("xT", [D, N], bf16, kind="ExternalInput")
    wq = nc.dram_tensor("wq", [D, 512], bf16, kind="ExternalInput")
    wk = nc.dram_tensor("wk", [D, 128], bf16, kind="ExternalInput")
    wv = nc.dram_tensor("wv", [D, 128], bf16, kind="ExternalInput")
    wo = nc.dram_tensor("wo", [512, D], bf16, kind="ExternalInput")
    msk = nc.dram_tensor("msk", [128, 128], bf16, kind="ExternalInput")
    iden = nc.dram_tensor("iden", [128, 128], bf16, kind="ExternalInput")
    part = nc.dram_tensor("part", [N, D], f32, kind="ExternalOutput")

    with tile.TileContext(nc) as tc:
        with (
            tc.tile_pool(name="sb", bufs=1) as sb,
            tc.tile_pool(name="ps", bufs=2, space="PSUM") as ps,
        ):
            # ---- input DMAs, ordered so compute can start early ----------
            msk_t = sb.tile([128, 128], bf16, tag="msk")
            nc.sync.dma_start(msk_t[:], msk.ap()[:])
            id_t = sb.tile([128, 128], bf16, tag="iden")
            nc.sync.dma_start(id_t[:], iden.ap()[:])
            wk_t = sb.tile([128, NT * 128], bf16, tag="wk")
            nc.sync.dma_start(
                wk_t[:].rearrange("p (t o) -> p t o", t=NT),
                wk.ap().rearrange("(t p) o -> p t o", p=128))
            wv_t = sb.tile([128, NT * 128], bf16, tag="wv")
            nc.sync.dma_start(
                wv_t[:].rearrange("p (t o) -> p t o", t=NT),
                wv.ap().rearrange("(t p) o -> p t o", p=128))
            xt = sb.tile([128, NT * N], bf16, tag="xt")
            xr = xt[:].rearrange("p (t n) -> p t n", t=NT)
            xsrc = xT.ap().rearrange("(t p) n -> p t n", p=128)
            wq_t = sb.tile([128, NT * 512], bf16, tag="wq")
            wo_t = sb.tile([128, 4 * D], bf16, tag="wo")
            for j in range(NCHUNK):
                nc.sync.dma_start(xr[:, :, j * CH:(j + 1) * CH],
                                  xsrc[:, :, j * CH:(j + 1) * CH])
                if j == 1:
                    nc.sync.dma_start(
                        wq_t[:].rearrange("p (t o) -> p t o", t=NT),
                        wq.ap().rearrange("(t p) o -> p t o", p=128))
            nc.sync.dma_start(
                wo_t[:].rearrange("p (t o) -> p t o", t=4),
                wo.ap().rearrange("(t p) o -> p t o", p=128))

            # ---- k/v projections + V transpose, chunk by chunk -----------
            kt2 = sb.tile([128, N], bf16, tag="kt2")
            v3 = sb.tile([128, 16 * 130], bf16, tag="v3")
            nc.vector.memset(v3[:], 1.0)
            for j in range(NCHUNK):
                psk = ps.tile([128, CH], f32, tag="ss")
                for t in range(NT):
                    nc.tensor.matmul(
                        psk[:], wk_t[:, t * 128:(t + 1) * 128],
                        xt[:, t * N + j * CH: t * N + (j + 1) * CH],
                        start=(t == 0), stop=(t == NT - 1))
                nc.scalar.activation(kt2[:, j * CH:(j + 1) * CH], psk[:], Copy)
                psv = ps.tile([128, CH], f32, tag="ss")
                for t in range(NT):
                    nc.tensor.matmul(
                        psv[:], wv_t[:, t * 128:(t + 1) * 128],
                        xt[:, t * N + j * CH: t * N + (j + 1) * CH],
                        start=(t == 0), stop=(t == NT - 1))
                vt_s = sb.tile([128, CH], bf16, tag="vt", bufs=2)
                nc.scalar.activation(vt_s[:], psv[:], Copy)
                for s4 in range(4):
                    mt = 4 * j + s4
                    pst = ps.tile([128, 128], bf16, tag="pa")
                    nc.tensor.transpose(pst[:], vt_s[:, s4 * 128:(s4 + 1) * 128],
                                        id_t[:])
                    nc.vector.tensor_copy(v3[:, mt * 130: mt * 130 + 64],
                                          pst[:, 0:64])
                    nc.vector.tensor_copy(v3[:, mt * 130 + 65: mt * 130 + 129],
                                          pst[:, 64:128])

            # ---- q projection for one chunk ------------------------------
            qt = sb.tile([128, 4 * N], bf16, tag="qt")   # [p, (pair a, n)]

            def qproj(ci):
                for a in range(4):
                    psq = ps.tile([128, CH], f32, tag="ss")
                    for t in range(NT):
                        nc.tensor.matmul(
                            psq[:],
                            wq_t[:, t * 512 + a * 128: t * 512 + (a + 1) * 128],
                            xt[:, t * N + ci * CH: t * N + (ci + 1) * CH],
                            start=(t == 0), stop=(t == NT - 1))
                    nc.scalar.activation(
                        qt[:, a * N + ci * CH: a * N + (ci + 1) * CH], psq[:],
                        Copy)

            # ---- attention for one chunk ---------------------------------
            an_tiles = {}

            def attention(ci):
                mt_hi = 4 * (ci + 1)
                for a in range(4):
                    pa_t = ps.tile([128, 2 * CH], f32, tag="pa")
                    for mt in range(mt_hi):
                        s = mt - 4 * ci
                        lo = 128 * s if s > 0 else 0
                        ss_t = ps.tile([128, 2 * CH], f32, tag="ss")
                        nc.tensor.matmul(
                            ss_t[:, lo:CH], kt2[0:64, mt * 128:(mt + 1) * 128],
                            qt[0:64, a * N + ci * CH + lo: a * N + (ci + 1) * CH],
                            start=True, stop=True)
                        nc.tensor.matmul(
                            ss_t[:, CH + lo:2 * CH],
                            kt2[64:128, mt * 128:(mt + 1) * 128],
                            qt[64:128, a * N + ci * CH + lo: a * N + (ci + 1) * CH],
                            start=True, stop=True)
                        pt = sb.tile([128, 2 * CH], bf16, tag="pt", bufs=4)
                        if lo == 0:
                            nc.scalar.activation(pt[:], ss_t[:], Exp, scale=0.125)
                        else:
                            ss3 = ss_t[:].rearrange("p (h q) -> p h q", h=2)
                            pt3 = pt[:].rearrange("p (h q) -> p h q", h=2)
                            nc.scalar.activation(pt3[:, :, lo:CH],
                                                 ss3[:, :, lo:CH], Exp,
                                                 scale=0.125)
                        if s >= 0:
                            nc.vector.tensor_mul(pt[:, lo:lo + 128],
                                                 pt[:, lo:lo + 128], msk_t[:])
                            nc.vector.tensor_mul(pt[:, CH + lo:CH + lo + 128],
                                                 pt[:, CH + lo:CH + lo + 128],
                                                 msk_t[:])
                        nc.tensor.matmul(
                            pa_t[0:65, lo:CH], v3[:, mt * 130: mt * 130 + 65],
                            pt[:, lo:CH], start=(mt == 0),
                            stop=(mt == mt_hi - 1), skip_group_check=True)
                        nc.tensor.matmul(
                            pa_t[0:65, CH + lo:2 * CH],
                            v3[:, mt * 130 + 65: mt * 130 + 130],
                            pt[:, CH + lo:2 * CH], start=(mt == 0),
                            stop=(mt == mt_hi - 1), skip_group_check=True)
                    # softmax denominators -> reciprocal -> broadcast
                    dsum = sb.tile([1, 2 * CH], f32, tag="dsum", bufs=4)
                    nc.vector.tensor_copy(dsum[:], pa_t[64:65, :])
                    rsum = sb.tile([1, 2 * CH], f32, tag="rsum", bufs=4)
                    nc.vector.reciprocal_approx_fast(rsum[:], dsum[:])
                    rb0 = sb.tile([128, CH], f32, tag="rb", bufs=4)
                    nc.gpsimd.partition_broadcast(rb0[:], rsum[0:1, 0:CH])
                    rb1 = sb.tile([128, CH], f32, tag="rb", bufs=4)
                    nc.gpsimd.partition_broadcast(rb1[:], rsum[0:1, CH:2 * CH])
                    an = sb.tile([128, CH], bf16, tag="an", bufs=8)
                    nc.vector.tensor_mul(an[0:64, :], pa_t[0:64, 0:CH],
                                         rb0[0:64, :])
                    nc.vector.tensor_copy(an[64:128, :], pa_t[0:64, CH:2 * CH])
                    nc.vector.tensor_mul(an[64:128, :], an[64:128, :],
                                         rb1[64:128, :])
                    an_tiles[(ci, a)] = an

            # ---- o_proj for one chunk ------------------------------------
            def oproj(ci):
                for nt in range(4):
                    for dc in range(4):
                        po = ps.tile([128, CH], f32, tag="ss")
                        for a in range(4):
                            nc.tensor.matmul(
                                po[:],
                                an_tiles[(ci, a)][:, nt * 128:(nt + 1) * 128],
                                wo_t[:, a * D + dc * CH: a * D + (dc + 1) * CH],
                                start=(a == 0), stop=(a == 3))
                        st = sb.tile([128, CH], f32, tag="st", bufs=3)
                        nc.vector.tensor_copy(st[:], po[:])
                        nc.sync.dma_start(
                            part.ap()[ci * CH + nt * 128: ci * CH + (nt + 1) * 128,
                                      dc * CH:(dc + 1) * CH],
                            st[:])

            # ---- interleaved schedule ------------------------------------
            qproj(0)
            attention(0)
            for ci in range(1, NCHUNK):
                qproj(ci)
                oproj(ci - 1)
                attention(ci)
            oproj(NCHUNK - 1)
    nc.compile()
    return nc


def _prep_in_maps(x, Wq, Wk, Wv, Wo):
    import jax.numpy as jnp

    def to_bf16(a):
        return np.asarray(jnp.asarray(np.asarray(a), dtype=jnp.bfloat16))

    # triangular mask for the 128x128 diagonal block: keep key i <= query j
    i = np.arange(128)[:, None]
    j = np.arange(128)[None, :]
    msk = (i <= j).astype(np.float32)
    iden = np.eye(128, dtype=np.float32)

    in_maps = []
    for c in range(N_CORES):
        b, g = c // 4, c % 4
        qh = [8 * g + a for a in range(8)]      # global q heads for this core
        # Wq columns reordered into pair chunks [head a | head a+4]
        wq_cols = []
        for a in range(4):
            wq_cols.append(np.arange(qh[a] * HD, (qh[a] + 1) * HD))
            wq_cols.append(np.arange(qh[a + 4] * HD, (qh[a + 4] + 1) * HD))
        wq_r = np.asarray(Wq)[:, np.concatenate(wq_cols)]
        wo_r = np.asarray(Wo)[np.concatenate(wq_cols), :]
        wk_s = np.asarray(Wk)[:, 2 * g * HD: (2 * g + 2) * HD]
        wv_s = np.asarray(Wv)[:, 2 * g * HD: (2 * g + 2) * HD]
        in_maps.append({
            "xT": to_bf16(np.asarray(x)[b].T),
            "wq": to_bf16(wq_r),
            "wk": to_bf16(wk_s),
            "wv": to_bf16(wv_s),
            "wo": to_bf16(wo_r),
            "msk": to_bf16(msk),
            "iden": to_bf16(iden),
        })
    return in_maps


def kernel(x, Wq, Wk, Wv, Wo, trace=False):
    if "nc" not in _CACHE:
        _CACHE["nc"] = _build()
    nc = _CACHE["nc"]
    in_maps = _prep_in_maps(x, Wq, Wk, Wv, Wo)
    res = bass_utils.run_bass_kernel_spmd(
        nc, in_maps, core_ids=list(range(N_CORES)), trace=trace)
    _CACHE["last_result"] = res
    out = np.zeros((B, N, D), np.float32)
    for c in range(N_CORES):
        out[c // 4] += res.results[c]["part"]
    return out


# revision 7
# speedup vs baseline: 1.5365x; 1.0001x over previous
"""GroupedQueryAttention forward on 8 Trainium2 NeuronCores (Bass/Tile).

Sharding (per spec hint): data-parallel over batch (B=2) x tensor-parallel
over KV-head groups (4 groups of 2 KV heads + their 8 query heads each).
Core c -> (batch b = c // 4, group g = c % 4).

Each core computes, for its batch element and its 8 query heads:
  qT/kT projections in transposed layout (lhsT = W, rhs = xT), V natural via
  on-chip PE transpose of vT; causal softmax without max-subtraction (scores
  are ~N(0,1) after the 1/sqrt(hd) scale, exp cannot overflow); the softmax
  denominator is produced by the same matmul as attn@V via a ones-column
  appended to V. o_proj is row-parallel: each core emits a full [N, D] fp32
  partial, and the host sums the 4 partials per batch element.

v2 structure (vs the v1 baseline):
  - per-chunk interleave: kv proj, then per 512-token chunk
    qproj(ci) -> attention(ci) -> [qproj(ci+1) overlap] -> o_proj(ci)
  - per (pair, mt): both kv-heads' score matmuls land in one 2-bank PSUM
    tile -> a single batched Exp; causal-diagonal tiles trim the dead
    query range out of scores/exp/attnV; mask multiply shrinks to the
    128x128 triangular block.
  - softmax denominators: reciprocal_approx_fast + GpSimd partition
    broadcast (replaces fp32 PE broadcast matmuls of v1).
All device compute is bf16 with fp32 PSUM accumulation.
"""

import numpy as np

import concourse.bass as bass  # noqa: F401  (import keeps engine registry warm)
import concourse.mybir as mybir
import concourse.tile as tile
from concourse import bacc, bass_utils

# Problem shape (hardcoded per contract).
B, N, D = 2, 2048, 2048
NUM_HEADS = 32
NUM_KV_HEADS = 8
HD = 64
G = NUM_HEADS // NUM_KV_HEADS
N_CORES = 8
NT = D // 128                # 16 contraction tiles
CH = 512
NCHUNK = N // CH             # 4

_CACHE = {}


def _build():
    nc = bacc.Bacc("TRN2", target_bir_lowering=False, debug=False,
                   num_devices=N_CORES)
    f32, bf16 = mybir.dt.float32, mybir.dt.bfloat16
    Exp = mybir.ActivationFunctionType.Exp

    xT = nc.dram_tensor("xT", [D, N], bf16, kind="ExternalInput")
    wq = nc.dram_tensor("wq", [D, 512], bf16, kind="ExternalInput")
    wk = nc.dram_tensor("wk", [D, 128], bf16, kind="ExternalInput")
    wv = nc.dram_tensor("wv", [D, 128], bf16, kind="ExternalInput")
    wo = nc.dram_tensor("wo", [512, D], bf16, kind="ExternalInput")
    msk = nc.dram_tensor("msk", [128, 128], bf16, kind="ExternalInput")
    iden = nc.dram_tensor("iden", [128, 128], bf16, kind="ExternalInput")
    part = nc.dram_tensor("part", [N, D], f32, kind="ExternalOutput")

    with tile.TileContext(nc) as tc:
        with (
            tc.tile_pool(name="sb", bufs=1) as sb,
            tc.tile_pool(name="ps", bufs=2, space="PSUM") as ps,
        ):
            # ---- input DMAs, ordered so compute can start early ----------
            msk_t = sb.tile([128, 128], bf16, tag="msk")
            nc.sync.dma_start(msk_t[:], msk.ap()[:])
            id_t = sb.tile([128, 128], bf16, tag="iden")
            nc.sync.dma_start(id_t[:], iden.ap()[:])
            wk_t = sb.tile([128, NT * 128], bf16, tag="wk")
            nc.sync.dma_start(
                wk_t[:].rearrange("p (t o) -> p t o", t=NT),
                wk.ap().rearrange("(t p) o -> p t o", p=128))
            xt = sb.tile([128, NT * N], bf16, tag="xt")
            xr = xt[:].rearrange("p (t n) -> p t n", t=NT)
            xsrc = xT.ap().rearrange("(t p) n -> p t n", p=128)
            wq_t = sb.tile([128, NT * 512], bf16, tag="wq")
            wo_t = sb.tile([128, 4 * D], bf16, tag="wo")
            wv_t = sb.tile([128, NT * 128], bf16, tag="wv")
            # chunk 0 split per contraction tile so the first k-proj matmuls
            # can start as soon as their slice lands
            for t in range(NT):
                nc.sync.dma_start(xr[:, t, 0:CH], xsrc[:, t, 0:CH])
            nc.sync.dma_start(
                wv_t[:].rearrange("p (t o) -> p t o", t=NT),
                wv.ap().rearrange("(t p) o -> p t o", p=128))
            for j in range(1, NCHUNK):
                nc.sync.dma_start(xr[:, :, j * CH:(j + 1) * CH],
                                  xsrc[:, :, j * CH:(j + 1) * CH])
                if j == 1:
                    nc.sync.dma_start(
                        wq_t[:].rearrange("p (t o) -> p t o", t=NT),
                        wq.ap().rearrange("(t p) o -> p t o", p=128))
            nc.sync.dma_start(
                wo_t[:].rearrange("p (t o) -> p t o", t=4),
                wo.ap().rearrange("(t p) o -> p t o", p=128))

            # ---- k/v projections + V transpose, chunk by chunk -----------
            kt2 = sb.tile([128, N], bf16, tag="kt2")
            v3 = sb.tile([128, 16 * 130], bf16, tag="v3")
            nc.vector.memset(v3[:], 1.0)
            for j in range(NCHUNK):
                psk = ps.tile([128, CH], f32, tag="ss")
                for t in range(NT):
                    nc.tensor.matmul(
                        psk[:], wk_t[:, t * 128:(t + 1) * 128],
                        xt[:, t * N + j * CH: t * N + (j + 1) * CH],
                        start=(t == 0), stop=(t == NT - 1))
                nc.vector.tensor_copy(kt2[:, j * CH:(j + 1) * CH], psk[:])
                psv = ps.tile([128, CH], f32, tag="ss")
                for t in range(NT):
                    nc.tensor.matmul(
                        psv[:], wv_t[:, t * 128:(t + 1) * 128],
                        xt[:, t * N + j * CH: t * N + (j + 1) * CH],
                        start=(t == 0), stop=(t == NT - 1))
                vt_s = sb.tile([128, CH], bf16, tag="vt", bufs=2)
                nc.vector.tensor_copy(vt_s[:], psv[:])
                for s4 in range(4):
                    mt = 4 * j + s4
                    pst = ps.tile([128, 128], bf16, tag="pa")
                    nc.tensor.transpose(pst[:], vt_s[:, s4 * 128:(s4 + 1) * 128],
                                        id_t[:])
                    nc.vector.tensor_copy(v3[:, mt * 130: mt * 130 + 64],
                                          pst[:, 0:64])
                    nc.vector.tensor_copy(v3[:, mt * 130 + 65: mt * 130 + 129],
                                          pst[:, 64:128])

            # ---- q projection for one chunk ------------------------------
            qt = sb.tile([128, 4 * N], bf16, tag="qt")   # [p, (pair a, n)]

            def qproj(ci):
                for a in range(4):
                    psq = ps.tile([128, CH], f32, tag="ss")
                    for t in range(NT):
                        nc.tensor.matmul(
                            psq[:],
                            wq_t[:, t * 512 + a * 128: t * 512 + (a + 1) * 128],
                            xt[:, t * N + ci * CH: t * N + (ci + 1) * CH],
                            start=(t == 0), stop=(t == NT - 1))
                    nc.vector.tensor_copy(
                        qt[:, a * N + ci * CH: a * N + (ci + 1) * CH], psq[:])

            # ---- attention for one chunk ---------------------------------
            an_tiles = {}

            def attention(ci):
                mt_hi = 4 * (ci + 1)
                LAG = 2
                for a in range(4):
                    pa_t = ps.tile([128, 2 * CH], f32, tag="pa")
                    pts = {}
                    los = {}

                    def score_stage(mt):
                        s = mt - 4 * ci
                        lo = 128 * s if s > 0 else 0
                        los[mt] = lo
                        ss_t = ps.tile([128, 2 * CH], f32, tag="ss")
                        nc.tensor.matmul(
                            ss_t[:, lo:CH], kt2[0:64, mt * 128:(mt + 1) * 128],
                            qt[0:64, a * N + ci * CH + lo: a * N + (ci + 1) * CH],
                            start=True, stop=True)
                        nc.tensor.matmul(
                            ss_t[:, CH + lo:2 * CH],
                            kt2[64:128, mt * 128:(mt + 1) * 128],
                            qt[64:128, a * N + ci * CH + lo: a * N + (ci + 1) * CH],
                            start=True, stop=True)
                        pt = sb.tile([128, 2 * CH], bf16, tag="pt", bufs=6)
                        if lo == 0:
                            nc.scalar.activation(pt[:], ss_t[:], Exp, scale=0.125)
                        else:
                            ss3 = ss_t[:].rearrange("p (h q) -> p h q", h=2)
                            pt3 = pt[:].rearrange("p (h q) -> p h q", h=2)
                            nc.scalar.activation(pt3[:, :, lo:CH],
                                                 ss3[:, :, lo:CH], Exp,
                                                 scale=0.125)
                        if s >= 0:
                            nc.vector.tensor_mul(pt[:, lo:lo + 128],
                                                 pt[:, lo:lo + 128], msk_t[:])
                            nc.vector.tensor_mul(pt[:, CH + lo:CH + lo + 128],
                                                 pt[:, CH + lo:CH + lo + 128],
                                                 msk_t[:])
                        pts[mt] = pt

                    def av_stage(mt):
                        lo = los[mt]
                        pt = pts.pop(mt)
                        nc.tensor.matmul(
                            pa_t[0:65, lo:CH], v3[:, mt * 130: mt * 130 + 65],
                            pt[:, lo:CH], start=(mt == 0),
                            stop=(mt == mt_hi - 1), skip_group_check=True)
                        nc.tensor.matmul(
                            pa_t[0:65, CH + lo:2 * CH],
                            v3[:, mt * 130 + 65: mt * 130 + 130],
                            pt[:, CH + lo:2 * CH], start=(mt == 0),
                            stop=(mt == mt_hi - 1), skip_group_check=True)

                    for mt in range(mt_hi + LAG):
                        if mt < mt_hi:
                            score_stage(mt)
                        if mt >= LAG:
                            av_stage(mt - LAG)
                    # softmax denominators -> reciprocal -> broadcast
                    dsum = sb.tile([1, 2 * CH], f32, tag="dsum", bufs=4)
                    nc.vector.tensor_copy(dsum[:], pa_t[64:65, :])
                    rsum = sb.tile([1, 2 * CH], f32, tag="rsum", bufs=4)
                    nc.vector.reciprocal_approx_fast(rsum[:], dsum[:])
                    rb = sb.tile([128, 2 * CH], f32, tag="rb", bufs=4)
                    nc.gpsimd.partition_broadcast(rb[:], rsum[:])
                    an = sb.tile([128, CH], bf16, tag="an", bufs=8)
                    nc.vector.tensor_mul(an[0:64, :], pa_t[0:64, 0:CH],
                                         rb[0:64, 0:CH])
                    nc.vector.tensor_copy(an[64:128, :], pa_t[0:64, CH:2 * CH])
                    nc.vector.tensor_mul(an[64:128, :], an[64:128, :],
                                         rb[64:128, CH:2 * CH])
                    an_tiles[(ci, a)] = an

            # ---- o_proj for one chunk ------------------------------------
            def oproj(ci):
                for nt in range(4):
                    for dc in range(4):
                        po = ps.tile([128, CH], f32, tag="ss")
                        for a in range(4):
                            nc.tensor.matmul(
                                po[:],
                                an_tiles[(ci, a)][:, nt * 128:(nt + 1) * 128],
                                wo_t[:, a * D + dc * CH: a * D + (dc + 1) * CH],
                                start=(a == 0), stop=(a == 3))
                        st = sb.tile([128, CH], f32, tag="st", bufs=3)
                        nc.vector.tensor_copy(st[:], po[:])
                        nc.sync.dma_start(
                            part.ap()[ci * CH + nt * 128: ci * CH + (nt + 1) * 128,
                                      dc * CH:(dc + 1) * CH],
                            st[:])

            # ---- interleaved schedule ------------------------------------
            qproj(0)
            attention(0)
            for ci in range(1, NCHUNK):
                qproj(ci)
                oproj(ci - 1)
                attention(ci)
            oproj(NCHUNK - 1)
    nc.compile()
    return nc


def _prep_in_maps(x, Wq, Wk, Wv, Wo):
    import jax.numpy as jnp

    def to_bf16(a):
        return np.asarray(jnp.asarray(np.asarray(a), dtype=jnp.bfloat16))

    # triangular mask for the 128x128 diagonal block: keep key i <= query j
    i = np.arange(128)[:, None]
    j = np.arange(128)[None, :]
    msk = (i <= j).astype(np.float32)
    iden = np.eye(128, dtype=np.float32)

    in_maps = []
    for c in range(N_CORES):
        b, g = c // 4, c % 4
        qh = [8 * g + a for a in range(8)]      # global q heads for this core
        # Wq columns reordered into pair chunks [head a | head a+4]
        wq_cols = []
        for a in range(4):
            wq_cols.append(np.arange(qh[a] * HD, (qh[a] + 1) * HD))
            wq_cols.append(np.arange(qh[a + 4] * HD, (qh[a + 4] + 1) * HD))
        wq_r = np.asarray(Wq)[:, np.concatenate(wq_cols)]
        wo_r = np.asarray(Wo)[np.concatenate(wq_cols), :]
        wk_s = np.asarray(Wk)[:, 2 * g * HD: (2 * g + 2) * HD]
        wv_s = np.asarray(Wv)[:, 2 * g * HD: (2 * g + 2) * HD]
        in_maps.append({
            "xT": to_bf16(np.asarray(x)[b].T),
            "wq": to_bf16(wq_r),
            "wk": to_bf16(wk_s),
            "wv": to_bf16(wv_s),
            "wo": to_bf16(wo_r),
            "msk": to_bf16(msk),
            "iden": to_bf16(iden),
        })
    return in_maps


def kernel(x, Wq, Wk, Wv, Wo, trace=False):
    if "nc" not in _CACHE:
        _CACHE["nc"] = _build()
    nc = _CACHE["nc"]
    in_maps = _prep_in_maps(x, Wq, Wk, Wv, Wo)
    res = bass_utils.run_bass_kernel_spmd(
        nc, in_maps, core_ids=list(range(N_CORES)), trace=trace)
    _CACHE["last_result"] = res
    out = np.zeros((B, N, D), np.float32)
    for c in range(N_CORES):
        out[c // 4] += res.results[c]["part"]
    return out


# revision 11
# speedup vs baseline: 1.5534x; 1.0109x over previous
"""GroupedQueryAttention forward on 8 Trainium2 NeuronCores (Bass/Tile).

Sharding (per spec hint): data-parallel over batch (B=2) x tensor-parallel
over KV-head groups (4 groups of 2 KV heads + their 8 query heads each).
Core c -> (batch b = c // 4, group g = c % 4).

Each core computes, for its batch element and its 8 query heads:
  qT/kT projections in transposed layout (lhsT = W, rhs = xT), V natural via
  on-chip PE transpose of vT; causal softmax without max-subtraction (scores
  are ~N(0,1) after the 1/sqrt(hd) scale, exp cannot overflow); the softmax
  denominator is produced by the same matmul as attn@V via a ones-column
  appended to V. o_proj is row-parallel: each core emits a full [N, D] fp32
  partial, and the host sums the 4 partials per batch element.

v2 structure (vs the v1 baseline):
  - per-chunk interleave: kv proj, then per 512-token chunk
    qproj(ci) -> attention(ci) -> [qproj(ci+1) overlap] -> o_proj(ci)
  - per (pair, mt): both kv-heads' score matmuls land in one 2-bank PSUM
    tile -> a single batched Exp; causal-diagonal tiles trim the dead
    query range out of scores/exp/attnV; mask multiply shrinks to the
    128x128 triangular block.
  - softmax denominators: reciprocal_approx_fast + GpSimd partition
    broadcast (replaces fp32 PE broadcast matmuls of v1).
All device compute is bf16 with fp32 PSUM accumulation.
"""

import numpy as np

import concourse.bass as bass  # noqa: F401  (import keeps engine registry warm)
import concourse.mybir as mybir
import concourse.tile as tile
from concourse import bacc, bass_utils

# Problem shape (hardcoded per contract).
B, N, D = 2, 2048, 2048
NUM_HEADS = 32
NUM_KV_HEADS = 8
HD = 64
G = NUM_HEADS // NUM_KV_HEADS
N_CORES = 8
NT = D // 128                # 16 contraction tiles
CH = 512
NCHUNK = N // CH             # 4

_CACHE = {}


def _build():
    nc = bacc.Bacc("TRN2", target_bir_lowering=False, debug=False,
                   num_devices=N_CORES)
    f32, bf16 = mybir.dt.float32, mybir.dt.bfloat16
    Exp = mybir.ActivationFunctionType.Exp

    xT = nc.dram_tensor("xT", [D, N], bf16, kind="ExternalInput")
    wq = nc.dram_tensor("wq", [D, 512], bf16, kind="ExternalInput")
    wk = nc.dram_tensor("wk", [D, 128], bf16, kind="ExternalInput")
    wv = nc.dram_tensor("wv", [D, 128], bf16, kind="ExternalInput")
    wo = nc.dram_tensor("wo", [512, D], bf16, kind="ExternalInput")
    msk = nc.dram_tensor("msk", [128, 128], bf16, kind="ExternalInput")
    iden = nc.dram_tensor("iden", [128, 128], bf16, kind="ExternalInput")
    part = nc.dram_tensor("part", [N, D], f32, kind="ExternalOutput")

    with tile.TileContext(nc) as tc:
        with (
            tc.tile_pool(name="sb", bufs=1) as sb,
            tc.tile_pool(name="ps", bufs=2, space="PSUM") as ps,
        ):
            # ---- input DMAs, ordered so compute can start early ----------
            msk_t = sb.tile([128, 128], bf16, tag="msk")
            nc.sync.dma_start(msk_t[:], msk.ap()[:])
            id_t = sb.tile([128, 128], bf16, tag="iden")
            nc.sync.dma_start(id_t[:], iden.ap()[:])
            wk_t = sb.tile([128, NT * 128], bf16, tag="wk")
            nc.sync.dma_start(
                wk_t[:].rearrange("p (t o) -> p t o", t=NT),
                wk.ap().rearrange("(t p) o -> p t o", p=128))
            xt = sb.tile([128, NT * N], bf16, tag="xt")
            xr = xt[:].rearrange("p (t n) -> p t n", t=NT)
            xsrc = xT.ap().rearrange("(t p) n -> p t n", p=128)
            wq_t = sb.tile([128, NT * 512], bf16, tag="wq")
            wo_t = sb.tile([128, 4 * D], bf16, tag="wo")
            wv_t = sb.tile([128, NT * 128], bf16, tag="wv")
            nc.sync.dma_start(xr[:, :, 0:CH], xsrc[:, :, 0:CH])
            nc.sync.dma_start(
                wv_t[:].rearrange("p (t o) -> p t o", t=NT),
                wv.ap().rearrange("(t p) o -> p t o", p=128))
            for j in range(1, NCHUNK):
                nc.sync.dma_start(xr[:, :, j * CH:(j + 1) * CH],
                                  xsrc[:, :, j * CH:(j + 1) * CH])
                if j == 1:
                    nc.sync.dma_start(
                        wq_t[:].rearrange("p (t o) -> p t o", t=NT),
                        wq.ap().rearrange("(t p) o -> p t o", p=128))
            nc.sync.dma_start(
                wo_t[:].rearrange("p (t o) -> p t o", t=4),
                wo.ap().rearrange("(t p) o -> p t o", p=128))

            # ---- k/v projections + V transpose, chunk by chunk -----------
            kt2 = sb.tile([128, N], bf16, tag="kt2")
            v3 = sb.tile([128, 16 * 130], bf16, tag="v3")
            nc.vector.memset(v3[:], 1.0)
            for j in range(NCHUNK):
                psk = ps.tile([128, CH], f32, tag="ss")
                for t in range(NT):
                    nc.tensor.matmul(
                        psk[:], wk_t[:, t * 128:(t + 1) * 128],
                        xt[:, t * N + j * CH: t * N + (j + 1) * CH],
                        start=(t == 0), stop=(t == NT - 1))
                nc.vector.tensor_copy(kt2[:, j * CH:(j + 1) * CH], psk[:])
                psv = ps.tile([128, CH], f32, tag="ss")
                for t in range(NT):
                    nc.tensor.matmul(
                        psv[:], wv_t[:, t * 128:(t + 1) * 128],
                        xt[:, t * N + j * CH: t * N + (j + 1) * CH],
                        start=(t == 0), stop=(t == NT - 1))
                vt_s = sb.tile([128, CH], bf16, tag="vt", bufs=2)
                nc.vector.tensor_copy(vt_s[:], psv[:])
                for s4 in range(4):
                    mt = 4 * j + s4
                    pst = ps.tile([128, 128], bf16, tag="pa")
                    nc.tensor.transpose(pst[:], vt_s[:, s4 * 128:(s4 + 1) * 128],
                                        id_t[:])
                    nc.vector.tensor_copy(v3[:, mt * 130: mt * 130 + 64],
                                          pst[:, 0:64])
                    nc.vector.tensor_copy(v3[:, mt * 130 + 65: mt * 130 + 129],
                                          pst[:, 64:128])

            # ---- q projection for one chunk ------------------------------
            qt = sb.tile([128, 4 * N], bf16, tag="qt")   # [p, (pair a, n)]

            def qproj_pair(ci, a):
                psq = ps.tile([128, CH], f32, tag="ss")
                for t in range(NT):
                    nc.tensor.matmul(
                        psq[:],
                        wq_t[:, t * 512 + a * 128: t * 512 + (a + 1) * 128],
                        xt[:, t * N + ci * CH: t * N + (ci + 1) * CH],
                        start=(t == 0), stop=(t == NT - 1))
                nc.vector.tensor_copy(
                    qt[:, a * N + ci * CH: a * N + (ci + 1) * CH], psq[:])

            def qproj(ci):
                for a in range(4):
                    qproj_pair(ci, a)

            # fill queue: deferred PE work (qproj / o_proj units) dripped
            # into the attention mt loops to cover ACT-bound exp stalls
            from collections import deque
            fills = deque()

            def fill_one():
                if fills:
                    fills.popleft()()

            # ---- attention for one chunk ---------------------------------
            an_tiles = {}

            def attention(ci):
                mt_hi = 4 * (ci + 1)
                LAG = 2
                for a in range(4):
                    pa_t = ps.tile([128, 2 * CH], f32, tag="pa")
                    pts = {}
                    los = {}

                    def score_stage(mt):
                        s = mt - 4 * ci
                        lo = 128 * s if s > 0 else 0
                        los[mt] = lo
                        ss_t = ps.tile([128, 2 * CH], f32, tag="ss")
                        nc.tensor.matmul(
                            ss_t[:, lo:CH], kt2[0:64, mt * 128:(mt + 1) * 128],
                            qt[0:64, a * N + ci * CH + lo: a * N + (ci + 1) * CH],
                            start=True, stop=True)
                        nc.tensor.matmul(
                            ss_t[:, CH + lo:2 * CH],
                            kt2[64:128, mt * 128:(mt + 1) * 128],
                            qt[64:128, a * N + ci * CH + lo: a * N + (ci + 1) * CH],
                            start=True, stop=True)
                        pt = sb.tile([128, 2 * CH], bf16, tag="pt", bufs=6)
                        if lo == 0:
                            nc.scalar.activation(pt[:], ss_t[:], Exp, scale=0.125)
                        else:
                            ss3 = ss_t[:].rearrange("p (h q) -> p h q", h=2)
                            pt3 = pt[:].rearrange("p (h q) -> p h q", h=2)
                            nc.scalar.activation(pt3[:, :, lo:CH],
                                                 ss3[:, :, lo:CH], Exp,
                                                 scale=0.125)
                        if s >= 0:
                            nc.vector.tensor_mul(pt[:, lo:lo + 128],
                                                 pt[:, lo:lo + 128], msk_t[:])
                            nc.vector.tensor_mul(pt[:, CH + lo:CH + lo + 128],
                                                 pt[:, CH + lo:CH + lo + 128],
                                                 msk_t[:])
                        pts[mt] = pt

                    def av_stage(mt):
                        lo = los[mt]
                        pt = pts.pop(mt)
                        nc.tensor.matmul(
                            pa_t[0:65, lo:CH], v3[:, mt * 130: mt * 130 + 65],
                            pt[:, lo:CH], start=(mt == 0),
                            stop=(mt == mt_hi - 1), skip_group_check=True)
                        nc.tensor.matmul(
                            pa_t[0:65, CH + lo:2 * CH],
                            v3[:, mt * 130 + 65: mt * 130 + 130],
                            pt[:, CH + lo:2 * CH], start=(mt == 0),
                            stop=(mt == mt_hi - 1), skip_group_check=True)

                    for mt in range(mt_hi + LAG):
                        if mt < mt_hi:
                            score_stage(mt)
                        if mt >= LAG:
                            av_stage(mt - LAG)
                        fill_one()
                    # softmax denominators -> reciprocal -> broadcast
                    dsum = sb.tile([1, 2 * CH], f32, tag="dsum", bufs=4)
                    nc.vector.tensor_copy(dsum[:], pa_t[64:65, :])
                    rsum = sb.tile([1, 2 * CH], f32, tag="rsum", bufs=4)
                    nc.vector.reciprocal_approx_fast(rsum[:], dsum[:])
                    rb = sb.tile([128, 2 * CH], f32, tag="rb", bufs=4)
                    nc.gpsimd.partition_broadcast(rb[:], rsum[:])
                    an = sb.tile([128, CH], bf16, tag="an", bufs=8)
                    nc.vector.tensor_mul(an[0:64, :], pa_t[0:64, 0:CH],
                                         rb[0:64, 0:CH])
                    nc.vector.tensor_copy(an[64:128, :], pa_t[0:64, CH:2 * CH])
                    nc.vector.tensor_mul(an[64:128, :], an[64:128, :],
                                         rb[64:128, CH:2 * CH])
                    an_tiles[(ci, a)] = an

            # ---- o_proj unit: one [128q, 512d] output block --------------
            def po_unit(ci, nt, dc):
                po = ps.tile([128, CH], f32, tag="ss")
                for a in range(4):
                    nc.tensor.matmul(
                        po[:],
                        an_tiles[(ci, a)][:, nt * 128:(nt + 1) * 128],
                        wo_t[:, a * D + dc * CH: a * D + (dc + 1) * CH],
                        start=(a == 0), stop=(a == 3))
                st = sb.tile([128, CH], f32, tag="st", bufs=4)
                nc.vector.tensor_copy(st[:], po[:])
                nc.sync.dma_start(
                    part.ap()[ci * CH + nt * 128: ci * CH + (nt + 1) * 128,
                              dc * CH:(dc + 1) * CH],
                    st[:])

            def push_oproj(ci):
                for nt in range(4):
                    for dc in range(4):
                        fills.append(lambda ci=ci, nt=nt, dc=dc:
                                     po_unit(ci, nt, dc))

            def push_qproj(ci):
                for a in range(4):
                    fills.append(lambda ci=ci, a=a: qproj_pair(ci, a))

            # ---- interleaved schedule ------------------------------------
            qproj(0)
            push_qproj(1)
            attention(0)
            for ci in range(1, NCHUNK):
                while fills:          # safety: qproj(ci) must precede
                    fill_one()        # attention(ci) emission
                push_oproj(ci - 1)
                if ci + 1 < NCHUNK:
                    push_qproj(ci + 1)
                attention(ci)
            while fills:
                fill_one()
            for nt in range(4):
                for dc in range(4):
                    po_unit(NCHUNK - 1, nt, dc)
    nc.compile()
    return nc


def _prep_in_maps(x, Wq, Wk, Wv, Wo):
    import jax.numpy as jnp

    def to_bf16(a):
        return np.asarray(jnp.asarray(np.asarray(a), dtype=jnp.bfloat16))

    # triangular mask for the 128x128 diagonal block: keep key i <= query j
    i = np.arange(128)[:, None]
    j = np.arange(128)[None, :]
    msk = (i <= j).astype(np.float32)
    iden = np.eye(128, dtype=np.float32)

    in_maps = []
    for c in range(N_CORES):
        b, g = c // 4, c % 4
        qh = [8 * g + a for a in range(8)]      # global q heads for this core
        # Wq columns reordered into pair chunks [head a | head a+4]
        wq_cols = []
        for a in range(4):
            wq_cols.append(np.arange(qh[a] * HD, (qh[a] + 1) * HD))
            wq_cols.append(np.arange(qh[a + 4] * HD, (qh[a + 4] + 1) * HD))
        wq_r = np.asarray(Wq)[:, np.concatenate(wq_cols)]
        wo_r = np.asarray(Wo)[np.concatenate(wq_cols), :]
        wk_s = np.asarray(Wk)[:, 2 * g * HD: (2 * g + 2) * HD]
        wv_s = np.asarray(Wv)[:, 2 * g * HD: (2 * g + 2) * HD]
        in_maps.append({
            "xT": to_bf16(np.asarray(x)[b].T),
            "wq": to_bf16(wq_r),
            "wk": to_bf16(wk_s),
            "wv": to_bf16(wv_s),
            "wo": to_bf16(wo_r),
            "msk": to_bf16(msk),
            "iden": to_bf16(iden),
        })
    return in_maps


def kernel(x, Wq, Wk, Wv, Wo, trace=False):
    if "nc" not in _CACHE:
        _CACHE["nc"] = _build()
    nc = _CACHE["nc"]
    in_maps = _prep_in_maps(x, Wq, Wk, Wv, Wo)
    res = bass_utils.run_bass_kernel_spmd(
        nc, in_maps, core_ids=list(range(N_CORES)), trace=trace)
    _CACHE["last_result"] = res
    out = np.zeros((B, N, D), np.float32)
    for c in range(N_CORES):
        out[c // 4] += res.results[c]["part"]
    return out
